# revision 1
# baseline (speedup 1.0000x reference)
"""ConcatAttention kernel for 8 Trainium2 NeuronCores.

Math: the reference computes softmax over scores[l, s] = (a_q[l] + a_k[s] + b)
/ sqrt(E) with a causal mask, where a_q = Q @ w_q and a_k = K @ w_k (the
"concat linear" score is additively separable).  Softmax over s is invariant
to terms constant in s, so the a_q[l] and bias terms cancel exactly:

    weights[l, s] = exp(a_k[s] / sqrt(E)) / sum_{s' <= l} exp(a_k[s'] / sqrt(E))
    out[l, :]     = (1 / den[l]) * sum_{s <= l} e_w[s] * V[s, :]

i.e. a cumulative weighted sum of V — O(L*E) work instead of O(L^2 * E).
Queries are not needed at all.

Sharding: batch*heads = 32 pairs; core c handles b = c // 4, heads
4*(c % 4) .. 4*(c % 4) + 3, so each core's K/V/out slices are contiguous
[2048, 4, 64] blocks in HBM after host-side slicing.

Engine placement (v4 — DMA-roofline oriented, pair-granular pipeline; a
"pair" is 2 chunks of 128 s-positions):
  - All HBM traffic rides the single SP HWDGE queue: pair-granular 256KB
    K/V loads interleaved (K leading), then 8 x 128KB pair stores (bf16).
    The DMA device is the binding resource.
  - a_k = K*w_k (tensor_tensor vs a fully materialized, unit-stride w_k
    tile) + two halves-folds per pair on the otherwise-idle GpSimd/Pool
    engine; the remaining 16-wide e-reduce runs on DVE.
  - exp on ACT writes e_w directly into the vp denominator columns; the
    per-pair vp multiply (DVE, f32r) broadcasts from those columns.
  - cumsum via PE matmuls: per-chunk totals into per-pair [2, W] PSUM
    waves, ACT-copied into two partition-padded tot_cat tiles (row
    blocks at 0/32/64/96 — the only legal AP start partitions), so each
    chunk's exclusive prefix is 1-2 matmuls vs padded su_cat masks and
    every dependency is pair-local.
  - reciprocal + normalize on DVE, emitting bf16 directly; norms lag the
    vp stream by NORM_LAG pairs so the PE->ACT->PE round trip never
    stalls the DVE spine.
"""

import numpy as np

B, L, H, E = 2, 2048, 16, 64
NCORES = 8
HPC = H * B // NCORES  # heads per core = 4
C = 16  # s-chunks
P = 128  # partitions per chunk
NP = C // 2  # pairs
W = HPC * E + HPC  # rhs width per chunk: 4*64 V-cols + 4 e_w cols = 260
SCALE = 1.0 / 8.0  # 1/sqrt(E)

# --- tunables (part of the build cache key) ---
OUT_BF16 = True
AK_POOL = True  # akmul + folds on Pool (else DVE)
FOLDS = 2  # halves-folds on Pool before the DVE e-reduce (0..2)
VP_POOL = ()  # pairs whose vp multiply runs on Pool
# load order: ("k", pair) / ("v", pair), K leading its pair's V enough for
# the a_k chain (Pool akmul+folds ~1us) to finish by the time V lands
LOAD_SEQ = [
    ("k", 0), ("v", 0), ("k", 1), ("v", 1), ("k", 2), ("v", 2),
    ("k", 3), ("k", 4), ("v", 3), ("k", 5), ("v", 4), ("k", 6),
    ("v", 5), ("k", 7), ("v", 6), ("v", 7),
]
NORM_LAG = 1  # norm of pair p is emitted after vp of pair p + NORM_LAG

_CACHE = {}


def _cfg():
    return (OUT_BF16, AK_POOL, FOLDS, tuple(VP_POOL), tuple(LOAD_SEQ), NORM_LAG)


def _build(reps=1):
    """Build the per-core module; reps>1 wraps the body in a hardware For_i
    loop (used only by the timing harness to amortize dispatch overhead)."""
    from contextlib import nullcontext

    import concourse.bacc as bacc
    import concourse.tile as tile
    import concourse.mybir as mybir

    f32 = mybir.dt.float32
    out_dt = mybir.dt.bfloat16 if OUT_BF16 else f32
    nc = bacc.Bacc("TRN2", target_bir_lowering=False, debug=False, num_devices=NCORES)

    k_in = nc.dram_tensor("k_in", [L, HPC, E], f32, kind="ExternalInput")
    v_in = nc.dram_tensor("v_in", [L, HPC, E], f32, kind="ExternalInput")
    wk_in = nc.dram_tensor("wk_in", [1, E], f32, kind="ExternalInput")
    out_d = nc.dram_tensor("out", [L, HPC, E], out_dt, kind="ExternalOutput")

    kv = k_in[:].rearrange("(c p) h e -> p c (h e)", p=P)  # [128, 16, 256]
    vv = v_in[:].rearrange("(c p) h e -> p c (h e)", p=P)
    ov = out_d[:].rearrange("(c p) h e -> p c (h e)", p=P)

    with tile.TileContext(nc) as tc:
        with (
            tc.tile_pool(name="consts", bufs=1) as consts,
            tc.tile_pool(name="big", bufs=1) as big,
            tc.tile_pool(name="small", bufs=1) as small,
            tc.tile_pool(name="pt", bufs=2, space="PSUM") as pt_pool,
            tc.tile_pool(name="pc", bufs=3, space="PSUM") as pc_pool,
        ):
            # f32r tiles: fp32 data streamed through the PE at full (1
            # cycle/row) rate; anything consumed by an f32r matmul must be
            # produced with f32r rounding, so these tiles are declared f32r.
            f32r = mybir.dt.float32r
            mult = mybir.AluOpType.mult
            addop = mybir.AluOpType.add

            # --- constants (one-time) ---
            # memset/affine_select cannot emit f32r, so masks are built in f32
            # scratch and copied (the copy applies f32r rounding; 0/1 exact).
            wk_sb = consts.tile([P, E], f32)
            wk_full = consts.tile([P, 2 * HPC, E], f32)  # wk per (c,h) of a pair
            scratch = consts.tile([P, P], f32)
            triu = consts.tile([P, P], f32r)  # triu[s, l] = 1 iff s <= l
            nc.vector.memset(scratch, 0.0)
            nc.gpsimd.affine_select(
                out=scratch,
                in_=scratch,
                compare_op=mybir.AluOpType.is_gt,
                fill=1.0,
                base=0,
                pattern=[[-1, P]],
                channel_multiplier=1,
            )
            nc.scalar.copy(out=triu, in_=scratch)
            nc.sync.dma_start(out=wk_sb, in_=wk_in[:].to_broadcast([P, E]))
            # materialized w_k across one pair's (c, h) width so the Pool
            # akmul reads unit-stride operands only (its broadcast-AP path
            # is slow on HW); built on DVE (ACT is busy with the exp-table
            # load and mask copies at start)
            nc.vector.tensor_copy(
                out=wk_full,
                in_=wk_sb[:].unsqueeze(1).to_broadcast([P, 2 * HPC, E]),
            )
            # Wave w (= pair w, 2 chunks) totals live at partitions
            # 32*(w%4)..+1 of tot_cat[w//4]; su_cat[t][32*b + r, c] = 1 iff
            # chunk 2*(4t + b) + r < c (r < 2), 0 in the padding rows.
            su_cat = []
            scat = []
            for t in range(2):
                sca = consts.tile([P, C], f32, name=f"scat{t}", tag=f"scat{t}")
                suc = consts.tile([P, C], f32r, name=f"sucat{t}", tag=f"sucat{t}")
                nc.vector.memset(sca, 0.0)
                scat.append(sca)
                su_cat.append(suc)
            for w in range(NP):
                sc = consts.tile([2, C], f32, name=f"scr_{w}", tag=f"scr_{w}")
                nc.vector.memset(sc, 0.0)
                nc.gpsimd.affine_select(
                    out=sc,
                    in_=sc,
                    compare_op=mybir.AluOpType.is_ge,
                    fill=1.0,
                    base=2 * w,
                    pattern=[[-1, C]],
                    channel_multiplier=1,
                )
                t, b = divmod(w, 4)
                nc.scalar.copy(out=scat[t][32 * b : 32 * b + 2, :], in_=sc)
            for t in range(2):
                nc.scalar.copy(out=su_cat[t], in_=scat[t])
            tot_cat = []
            scrw = consts.tile([P, W], f32)
            nc.vector.memset(scrw, 0.0)
            for t in range(2):
                tcat = consts.tile([P, W], f32r, name=f"tcat{t}", tag=f"tcat{t}")
                nc.scalar.copy(out=tcat, in_=scrw)
                tot_cat.append(tcat)
            sel = consts.tile([P, 3], f32r)  # col 1 ones: row-r selector
            scrsel = consts.tile([P, 3], f32)
            nc.vector.memset(scrsel, 0.0)
            nc.vector.memset(scrsel[:, 1:2], 1.0)
            nc.scalar.copy(out=sel, in_=scrsel)

            loop = tc.For_i(0, reps, 1) if reps > 1 else nullcontext()
            with loop:
                # --- working tiles ---
                k_t = big.tile([P, C, HPC * E], f32)
                v_t = big.tile([P, C, HPC * E], f32)
                t1 = big.tile([P, C, HPC * E], f32)
                t1f = big.tile([P, C, HPC * (E // 2)], f32)
                t1g = big.tile([P, C, HPC * (E // 4)], f32)
                vp = big.tile([P, C, W], f32r)
                o_t = big.tile([P, C, HPC * E], out_dt)
                a_k = small.tile([P, C * HPC], f32)
                r_all = small.tile([P, C * HPC], f32)
                ps_tot = [None] * NP

                def load_k(p):
                    cs = slice(2 * p, 2 * p + 2)
                    nc.sync.dma_start(out=k_t[:, cs, :], in_=kv[:, cs, :])

                def load_v(p):
                    cs = slice(2 * p, 2 * p + 2)
                    nc.sync.dma_start(out=v_t[:, cs, :], in_=vv[:, cs, :])

                dve_chain = []
                red_in_of = {}

                def pool_head(p):
                    """akmul (+ folds) for pair p on the Pool engine."""
                    cs = slice(2 * p, 2 * p + 2)
                    kg = k_t[:, cs, :].rearrange("p c (h e) -> p c h e", e=E)
                    tg = t1[:, cs, :].rearrange("p c (h e) -> p c h e", e=E)
                    wkg = wk_full[:].rearrange("p (c h) e -> p c h e", h=HPC)
                    ak_eng = nc.gpsimd if AK_POOL else nc.vector
                    ak = ak_eng.tensor_tensor(out=tg, in0=kg, in1=wkg, op=mult)
                    if not AK_POOL:
                        dve_chain.append(ak)
                    red_in = tg
                    width = E
                    for f in range(FOLDS):
                        width //= 2
                        dst = (t1f if f == 0 else t1g)[:, cs, :].rearrange(
                            "p c (h e) -> p c h e", e=width
                        )
                        fold_eng = nc.gpsimd if AK_POOL else nc.vector
                        fd = fold_eng.tensor_tensor(
                            out=dst,
                            in0=red_in[:, :, :, 0:width],
                            in1=red_in[:, :, :, width : 2 * width],
                            op=addop,
                        )
                        if not AK_POOL:
                            dve_chain.append(fd)
                        red_in = dst
                    red_in_of[p] = red_in

                def red_exp(p):
                    """e-reduce (DVE) then e_w = exp(a_k/8) into the vp
                    denominator columns (ACT)."""
                    cs = slice(2 * p, 2 * p + 2)
                    hs = slice(2 * p * HPC, (2 * p + 2) * HPC)
                    dve_chain.append(nc.vector.tensor_reduce(
                        out=a_k[:, hs].rearrange("p (c h) -> p c h", h=HPC),
                        in_=red_in_of[p],
                        axis=mybir.AxisListType.X,
                        op=addop,
                    ))
                    nc.scalar.activation(
                        out=vp[:, cs, HPC * E : W],
                        in_=a_k[:, hs].rearrange("p (c h) -> p c h", h=HPC),
                        func=mybir.ActivationFunctionType.Exp,
                        scale=SCALE,
                    )

                def vp_pair(p):
                    """vp = [V * e_w] for the pair's two chunks (e_w is read
                    back from the den columns the exp wrote)."""
                    cs = slice(2 * p, 2 * p + 2)
                    ew_b = (
                        vp[:, cs, HPC * E : W]
                        .unsqueeze(3)
                        .to_broadcast([P, 2, HPC, E])
                    )
                    eng = nc.gpsimd if p in VP_POOL else nc.vector
                    v = eng.tensor_tensor(
                        out=vp[:, cs, 0 : HPC * E].rearrange(
                            "p c (h e) -> p c h e", e=E
                        ),
                        in0=v_t[:, cs, :].rearrange("p c (h e) -> p c h e", e=E),
                        in1=ew_b,
                        op=mult,
                    )
                    if p not in VP_POOL:
                        dve_chain.append(v)

                BANK = 512  # f32 elements per PSUM bank
                psc_of = {}  # pair -> psum tile

                def total_tri(c):
                    """wave-total row + tri matmul for chunk c."""
                    w, r = divmod(c, 2)
                    if r == 0:
                        ps_tot[w] = pt_pool.tile(
                            [2, W], f32, name="ps_tot", tag="ptot"
                        )
                    nc.tensor.matmul(
                        ps_tot[w][:, :],
                        sel[:, 1 - r : 3 - r],
                        vp[:, c, :],
                        start=(r == 0),
                        stop=(r == 1),
                    )
                    if r == 1:
                        t, b = divmod(w, 4)
                        nc.scalar.copy(
                            out=tot_cat[t][32 * b : 32 * b + 2, :], in_=ps_tot[w]
                        )
                    cp, j = divmod(c, 2)
                    if j == 0:
                        psc_of[cp] = pc_pool.tile(
                            [P, 2, BANK], f32, name="psc", tag="psc"
                        )
                    blk = psc_of[cp][:, j, 0:W]
                    nc.tensor.matmul(
                        blk, triu[:, :], vp[:, c, :], start=True, stop=(c == 0)
                    )

                def prefixes(c):
                    """exclusive-prefix matmuls for chunk c against the padded
                    wave-total stacks (emitted after the wave copy so the
                    in-order PE queue never waits on a matmul behind it)."""
                    if c == 0:
                        return
                    cp, j = divmod(c, 2)
                    blk = psc_of[cp][:, j, 0:W]
                    last_w = (c - 1) // 2  # last wave with any prefix bit
                    for t in range(2):
                        if last_w < 4 * t:
                            break
                        b = min(last_w - 4 * t, 3)
                        kk = 32 * b + 2
                        nc.tensor.matmul(
                            blk,
                            su_cat[t][0:kk, c : c + 1].to_broadcast([kk, P]),
                            tot_cat[t][0:kk, :],
                            start=False,
                            stop=(t == 1 or last_w < 4),
                        )

                def norm_pair(cp):
                    """One reciprocal of the den columns and one broadcast
                    multiply to normalize; then store the pair."""
                    psc = psc_of.pop(cp)
                    c0 = 2 * cp
                    hs = slice(c0 * HPC, (c0 + 2) * HPC)
                    dve_chain.append(nc.vector.reciprocal(
                        out=r_all[:, hs].rearrange("p (c h) -> p c h", h=HPC),
                        in_=psc[:, :, HPC * E : W],
                    ))
                    r_b = (
                        r_all[:, hs]
                        .rearrange("p (c h) -> p c h", h=HPC)
                        .unsqueeze(3)
                        .to_broadcast([P, 2, HPC, E])
                    )
                    dve_chain.append(nc.vector.tensor_tensor(
                        out=o_t[:, c0 : c0 + 2, :].rearrange(
                            "p c (h e) -> p c h e", e=E
                        ),
                        in0=psc[:, :, 0 : HPC * E].rearrange(
                            "p c (h e) -> p c h e", e=E
                        ),
                        in1=r_b,
                        op=mult,
                    ))
                    cs = slice(c0, c0 + 2)
                    nc.sync.dma_start(out=ov[:, cs, :], in_=o_t[:, cs, :])

                for kind, p in LOAD_SEQ:
                    (load_k if kind == "k" else load_v)(p)

                # Pool runs ahead of the DVE stream (its FIFO paces on the K
                # loads); each pair's reduce is pulled ahead of the previous
                # pair's vp, and norms lag NORM_LAG pairs so the PE->ACT->PE
                # round trip of a pair never stalls the DVE spine.
                for p in range(NP):
                    pool_head(p)
                red_exp(0)
                normed = 0
                for p in range(NP):
                    vp_pair(p)
                    if p + 1 < NP:
                        red_exp(p + 1)
                    total_tri(2 * p)
                    total_tri(2 * p + 1)
                    prefixes(2 * p)
                    prefixes(2 * p + 1)
                    while normed <= p - NORM_LAG:
                        norm_pair(normed)
                        normed += 1
                while normed < NP:
                    norm_pair(normed)
                    normed += 1
                # Pin the static DVE order to the dataflow order
                for a, b in zip(dve_chain, dve_chain[1:]):
                    tile.add_dep_helper(
                        b.ins, a.ins, sync=False, reason="dve pipeline order"
                    )

    nc.compile()
    return nc


def _get_nc(reps=1):
    key = ("nc", reps, _cfg())
    if key not in _CACHE:
        _CACHE[key] = _build(reps)
    return _CACHE[key]


def _shard(inputs):
    keys = np.asarray(inputs["keys"], dtype=np.float32)
    values = np.asarray(inputs["values"], dtype=np.float32)
    w_score = np.asarray(inputs["w_score"], dtype=np.float32)
    wk = np.ascontiguousarray(w_score[E : 2 * E].reshape(1, E))
    in_maps = []
    for c in range(NCORES):
        b = c // (NCORES // B)
        h0 = HPC * (c % (NCORES // B))
        in_maps.append(
            {
                "k_in": np.ascontiguousarray(keys[b, :, h0 : h0 + HPC, :]),
                "v_in": np.ascontiguousarray(values[b, :, h0 : h0 + HPC, :]),
                "wk_in": wk,
            }
        )
    return in_maps


def _gather(results):
    out = np.empty((B, L, H, E), dtype=np.float32)
    for c in range(NCORES):
        b = c // (NCORES // B)
        h0 = HPC * (c % (NCORES // B))
        out[b, :, h0 : h0 + HPC, :] = np.asarray(results[c]["out"]).astype(
            np.float32
        )
    return out


def _run_sharded(inputs, reps=1, **kwargs):
    from concourse.bass_utils import run_bass_kernel_spmd

    nc = _get_nc(reps)
    in_maps = _shard(inputs)
    res = run_bass_kernel_spmd(nc, in_maps, core_ids=list(range(NCORES)), **kwargs)
    return res


def kernel(**inputs) -> np.ndarray:
    res = _run_sharded(inputs)
    return _gather(res.results)



# revision 60
# speedup vs baseline: 1.3229x; 1.3229x over previous
"""ConcatAttention kernel for 8 Trainium2 NeuronCores.

Math: the reference computes softmax over scores[l, s] = (a_q[l] + a_k[s] + b)
/ sqrt(E) with a causal mask, where a_q = Q @ w_q and a_k = K @ w_k (the
"concat linear" score is additively separable).  Softmax over s is invariant
to terms constant in s, so the a_q[l] and bias terms cancel exactly:

    weights[l, s] = exp(a_k[s] / sqrt(E)) / sum_{s' <= l} exp(a_k[s'] / sqrt(E))
    out[l, :]     = (1 / den[l]) * sum_{s <= l} e_w[s] * V[s, :]

i.e. a cumulative weighted sum of V — O(L*E) work instead of O(L^2 * E).
Queries are not needed at all.

Sharding: batch*heads = 32 pairs; core c handles b = c // 4, heads
4*(c % 4) .. 4*(c % 4) + 3, so each core's K/V/out slices are contiguous
[2048, 4, 64] blocks in HBM after host-side slicing.

Engine placement (v5 — latency/op-count oriented; a "pair" is 2 chunks of
128 s-positions, a "group" is GRAN=2 pairs).  HW findings that shaped it:
the single SP HWDGE queue streams the 5MB at near full bandwidth (~15us)
and hides under compute, HW per-instruction overhead dwarfs the cost
model's, and issuing DMA from ACT/Pool stalls those engines' SEQs.
  - All HBM traffic rides the single SP HWDGE queue: pair-granular 256KB
    K/V loads interleaved (K leading), stores batched 4 pairs / 512KB.
  - a_k = K*w_k on Pool (vs a materialized unit-stride w_k tile); the
    full 64-wide e-reduce runs on DVE per group (folds on Pool lose to
    the extra cross-engine hops on HW).
  - exp on ACT writes e_w into the vp denominator columns per group; the
    group vp multiply (DVE, f32r) broadcasts from those columns.
  - cumsum via PE matmuls: per-chunk totals into per-pair [2, W] PSUM
    waves, ACT-copied into two partition-padded tot_cat tiles (row
    blocks at 0/32/64/96 — the only legal AP start partitions), so each
    chunk's exclusive prefix is 1-2 matmuls vs padded su_cat masks and
    every dependency is pair-local.
  - reciprocal + normalize on DVE, emitting bf16 directly; norms lag the
    vp stream by NORM_LAG pairs so the PE->ACT->PE round trip never
    stalls the DVE spine.
  - the For_i body is unrolled 3x with rotated tile slots (t1/vp share
    one slot set — t1 dies at the e-reduce) so consecutive iterations
    overlap deeply; per-phase tot_cat copies avoid WAR serialization.
Measured: 35.7us (v4) -> ~20.3us/iter on HW (rel err 2.6e-3, bf16 out).
"""

import numpy as np

B, L, H, E = 2, 2048, 16, 64
NCORES = 8
HPC = H * B // NCORES  # heads per core = 4
C = 16  # s-chunks
P = 128  # partitions per chunk
NP = C // 2  # pairs
W = HPC * E + HPC  # rhs width per chunk: 4*64 V-cols + 4 e_w cols = 260
SCALE = 1.0 / 8.0  # 1/sqrt(E)

# --- tunables (part of the build cache key); defaults = best measured ---
OUT_BF16 = True
AK_POOL = True  # akmul on Pool (else DVE)
FOLDS = 0  # halves-folds on Pool before the DVE e-reduce (0..2); 0 won on
# HW: the extra Pool->DVE hops cost more than the wider DVE reduce
VP_POOL = ()  # vp groups whose multiply runs on Pool (none: Pool TT slow)
NORM_LAG = 1  # norm of pair p is emitted after vp of pair p + NORM_LAG
# DMA routing/grouping: TRN2 has two HWDGE queues (SP="sp", ACT="act") and
# the Pool SWDGE ("pool").  ACT/Pool issue measured slower on HW (SEQ
# head-of-line blocking); everything rides SP.
K_ENG = "sp"
V_ENG = "sp"
ST_ENG = "sp"
LOAD_GROUP = 1  # pairs per K/V load DMA (pair-granular pipelines best)
ST_GROUP = 4  # pairs per store DMA (stores gate nothing; batch them)
UNROLL = 3  # iterations per For_i body (cross-iteration overlap)
BUFS = 3  # slot rotation depth for the working pools
GRAN = 2  # pairs per reduce/exp/vp instruction: fewer, larger ops on the
# DVE/ACT spine (HW per-instruction overhead dominates small ops)
RED_POOL = False  # (Pool tensor_reduce can't do free-axis reductions)
AK_GRAN = 1  # pairs per akmul instruction (2 delays the a_k chain head)
TUNED_SEQ = True  # use the single-queue tuned K/V load interleave
SEQ_VARIANT = 0  # which tuned interleave (see SEQS in _build)
WAVE_BATCH = 1  # waves per tot_cat copy (2 delays psc completion; keep 1)
EW_FULL = False  # materialized e_w (broadcast reads are not a bottleneck)
TV_SHARE = True  # t1 and vp rotate through one shared slot set (SBUF save
# that makes UNROLL=3 fit; t1 dies at the e-reduce, before vp is written)
TV_BUFS = 4  # slots for the shared t1/vp tag
OT_BUFS = 2  # o_t slots (stores drain fast; 2 suffice, saves SBUF)
DMA_SCRATCH = 4096  # dynamic-DMA scratchpad bytes/partition (SBUF)
DIAG_SMALL_IO = False  # diagnostic: load only pair 0, compute reads it for
# every pair (wrong results, right instruction mix) — isolates compute time

_CACHE = {}


def _cfg():
    return (OUT_BF16, AK_POOL, FOLDS, tuple(VP_POOL), NORM_LAG,
            K_ENG, V_ENG, ST_ENG, LOAD_GROUP, ST_GROUP, UNROLL, BUFS,
            DIAG_SMALL_IO, GRAN, RED_POOL, TV_SHARE, TV_BUFS, OT_BUFS,
            DMA_SCRATCH, EW_FULL, AK_GRAN, TUNED_SEQ, WAVE_BATCH, SEQ_VARIANT)


def _build(reps=1):
    """Build the per-core module; reps>1 wraps the body in a hardware For_i
    loop (used only by the timing harness to amortize dispatch overhead)."""
    from contextlib import nullcontext

    import concourse.bacc as bacc
    import concourse.tile as tile
    import concourse.mybir as mybir

    f32 = mybir.dt.float32
    out_dt = mybir.dt.bfloat16 if OUT_BF16 else f32
    nc = bacc.Bacc(
        "TRN2",
        target_bir_lowering=False,
        debug=False,
        num_devices=NCORES,
        dynamic_dma_scratch_size=DMA_SCRATCH,
    )

    k_in = nc.dram_tensor("k_in", [L, HPC, E], f32, kind="ExternalInput")
    v_in = nc.dram_tensor("v_in", [L, HPC, E], f32, kind="ExternalInput")
    wk_in = nc.dram_tensor("wk_in", [1, E], f32, kind="ExternalInput")
    out_d = nc.dram_tensor("out", [L, HPC, E], out_dt, kind="ExternalOutput")

    kv = k_in[:].rearrange("(c p) h e -> p c (h e)", p=P)  # [128, 16, 256]
    vv = v_in[:].rearrange("(c p) h e -> p c (h e)", p=P)
    ov = out_d[:].rearrange("(c p) h e -> p c (h e)", p=P)

    with tile.TileContext(nc) as tc:
        with (
            tc.tile_pool(name="consts", bufs=1) as consts,
            tc.tile_pool(name="big", bufs=BUFS) as big,
            tc.tile_pool(name="small", bufs=BUFS) as small,
            tc.tile_pool(name="pt", bufs=2, space="PSUM") as pt_pool,
            tc.tile_pool(name="pc", bufs=3, space="PSUM") as pc_pool,
        ):
            # f32r tiles: fp32 data streamed through the PE at full (1
            # cycle/row) rate; anything consumed by an f32r matmul must be
            # produced with f32r rounding, so these tiles are declared f32r.
            f32r = mybir.dt.float32r
            mult = mybir.AluOpType.mult
            addop = mybir.AluOpType.add

            # --- constants (one-time) ---
            # memset/affine_select cannot emit f32r, so masks are built in f32
            # scratch and copied (the copy applies f32r rounding; 0/1 exact).
            wk_sb = consts.tile([P, E], f32)
            # wk materialized across one akmul group's (c, h) width
            wk_full = consts.tile([P, 2 * AK_GRAN * HPC, E], f32)
            scratch = consts.tile([P, P], f32)
            triu = consts.tile([P, P], f32r)  # triu[s, l] = 1 iff s <= l
            nc.vector.memset(scratch, 0.0)
            nc.gpsimd.affine_select(
                out=scratch,
                in_=scratch,
                compare_op=mybir.AluOpType.is_gt,
                fill=1.0,
                base=0,
                pattern=[[-1, P]],
                channel_multiplier=1,
            )
            nc.scalar.copy(out=triu, in_=scratch)
            nc.sync.dma_start(out=wk_sb, in_=wk_in[:].to_broadcast([P, E]))
            # materialized w_k across one pair's (c, h) width so the Pool
            # akmul reads unit-stride operands only (its broadcast-AP path
            # is slow on HW); built on DVE (ACT is busy with the exp-table
            # load and mask copies at start)
            nc.vector.tensor_copy(
                out=wk_full,
                in_=wk_sb[:].unsqueeze(1).to_broadcast([P, 2 * AK_GRAN * HPC, E]),
            )
            # Block w (WAVE_BATCH waves = RPB chunks) totals live at
            # partitions 32*(w % BPT)..+RPB of tot_cat[w // BPT];
            # su_cat[t][32*b + r, c] = 1 iff chunk RPB*(BPT*t + b) + r < c
            # (r < RPB), 0 in the padding rows.
            RPB = 2 * WAVE_BATCH  # chunk rows per block
            NBLK = C // RPB
            BPT = NBLK // 2  # blocks per table
            su_cat = []
            scat = []
            for t in range(2):
                sca = consts.tile([P, C], f32, name=f"scat{t}", tag=f"scat{t}")
                suc = consts.tile([P, C], f32r, name=f"sucat{t}", tag=f"sucat{t}")
                nc.vector.memset(sca, 0.0)
                scat.append(sca)
                su_cat.append(suc)
            for w in range(NBLK):
                sc = consts.tile([RPB, C], f32, name=f"scr_{w}", tag=f"scr_{w}")
                nc.vector.memset(sc, 0.0)
                nc.gpsimd.affine_select(
                    out=sc,
                    in_=sc,
                    compare_op=mybir.AluOpType.is_ge,
                    fill=1.0,
                    base=RPB * w,
                    pattern=[[-1, C]],
                    channel_multiplier=1,
                )
                t, b = divmod(w, BPT)
                nc.scalar.copy(out=scat[t][32 * b : 32 * b + RPB, :], in_=sc)
            for t in range(2):
                nc.scalar.copy(out=su_cat[t], in_=scat[t])
            # One tot_cat set per unrolled phase (the wave rows are
            # rewritten every iteration; sharing would WAR-serialize the
            # phases).  Padding rows are zeroed once here.
            scrw = consts.tile([P, W], f32)
            nc.vector.memset(scrw, 0.0)
            tot_cat_sets = []
            for ph in range(max(UNROLL, 1)):
                tset = []
                for t in range(2):
                    tcat = consts.tile(
                        [P, W], f32r, name=f"tcat{ph}_{t}", tag=f"tcat{ph}_{t}"
                    )
                    nc.scalar.copy(out=tcat, in_=scrw)
                    tset.append(tcat)
                tot_cat_sets.append(tset)
            # row-r selector: ones at col RPB-1 of [P, 2*RPB-1]; the slice
            # sel[:, RPB-1-r : 2*RPB-1-r] puts the ones-column at position r
            sel = consts.tile([P, 2 * RPB - 1], f32r)
            scrsel = consts.tile([P, 2 * RPB - 1], f32)
            nc.vector.memset(scrsel, 0.0)
            nc.vector.memset(scrsel[:, RPB - 1 : RPB], 1.0)
            nc.scalar.copy(out=sel, in_=scrsel)

            un = UNROLL if reps >= UNROLL and reps > 1 else 1
            # non-divisible reps run floor(reps/un)*un iterations; the rep
            # counts used for slope timing make the shortfall negligible
            dve_chain = []

            def body(phase):
                tot_cat = tot_cat_sets[phase % max(UNROLL, 1)]
                # --- working tiles (per-phase; slots rotate across the
                # unrolled phases and the For_i back edge) ---
                k_t = big.tile([P, C, HPC * E], f32, name="k_t")
                v_t = big.tile([P, C, HPC * E], f32, name="v_t")
                # t1 is dead after the e-reduce, long before vp is written;
                # TV_SHARE rotates both through one slot set to fit UNROLL=3
                tv = {"tag": "tv", "bufs": TV_BUFS} if TV_SHARE else {}
                t1 = big.tile([P, C, HPC * E], f32, name="t1", **tv)
                t1f = (
                    big.tile([P, C, HPC * (E // 2)], f32, name="t1f")
                    if FOLDS >= 1 else None
                )
                t1g = (
                    big.tile([P, C, HPC * (E // 4)], f32, name="t1g")
                    if FOLDS >= 2 else None
                )
                ew_f = (
                    big.tile([P, C, HPC * E], f32, name="ew_f", **tv)
                    if EW_FULL else None
                )
                vp = big.tile([P, C, W], f32r, name="vp", **tv)
                o_t = big.tile(
                    [P, C, HPC * E], out_dt, name="o_t",
                    **({"bufs": OT_BUFS} if OT_BUFS else {}),
                )
                a_k = small.tile([P, C * HPC], f32, name="a_k")
                r_all = small.tile([P, C * HPC], f32, name="r_all")
                ps_tot = [None] * NP

                dma_eng = {"sp": nc.sync, "act": nc.scalar, "pool": nc.gpsimd}

                def load_k(g):
                    cs = slice(2 * LOAD_GROUP * g, 2 * LOAD_GROUP * (g + 1))
                    dma_eng[K_ENG].dma_start(out=k_t[:, cs, :], in_=kv[:, cs, :])

                def load_v(g):
                    cs = slice(2 * LOAD_GROUP * g, 2 * LOAD_GROUP * (g + 1))
                    dma_eng[V_ENG].dma_start(out=v_t[:, cs, :], in_=vv[:, cs, :])

                def pool_head(a):
                    """akmul (+ folds) for AK_GRAN pairs on the Pool engine."""
                    nch = 2 * AK_GRAN
                    cs = slice(nch * a, nch * (a + 1))
                    ks = slice(0, nch) if DIAG_SMALL_IO else cs
                    kg = k_t[:, ks, :].rearrange("p c (h e) -> p c h e", e=E)
                    tg = t1[:, cs, :].rearrange("p c (h e) -> p c h e", e=E)
                    wkg = wk_full[:].rearrange("p (c h) e -> p c h e", h=HPC)
                    ak_eng = nc.gpsimd if AK_POOL else nc.vector
                    ak = ak_eng.tensor_tensor(out=tg, in0=kg, in1=wkg, op=mult)
                    if not AK_POOL:
                        dve_chain.append(ak)
                    red_in = tg
                    width = E
                    for f in range(FOLDS):
                        width //= 2
                        dst = (t1f if f == 0 else t1g)[:, cs, :].rearrange(
                            "p c (h e) -> p c h e", e=width
                        )
                        fold_eng = nc.gpsimd if AK_POOL else nc.vector
                        fd = fold_eng.tensor_tensor(
                            out=dst,
                            in0=red_in[:, :, :, 0:width],
                            in1=red_in[:, :, :, width : 2 * width],
                            op=addop,
                        )
                        if not AK_POOL:
                            dve_chain.append(fd)
                        red_in = dst

                red_tile = {0: t1, 1: t1f, 2: t1g}[FOLDS]
                red_w = E >> FOLDS

                def red_exp(g):
                    """e-reduce (DVE) then e_w = exp(a_k/8) into the vp
                    denominator columns (ACT), for GRAN pairs."""
                    nch = 2 * GRAN
                    cs = slice(nch * g, nch * (g + 1))
                    hs = slice(nch * g * HPC, nch * (g + 1) * HPC)
                    red_eng = nc.gpsimd if RED_POOL else nc.vector
                    red = red_eng.tensor_reduce(
                        out=a_k[:, hs].rearrange("p (c h) -> p c h", h=HPC),
                        in_=red_tile[:, cs, :].rearrange(
                            "p c (h e) -> p c h e", e=red_w
                        ),
                        axis=mybir.AxisListType.X,
                        op=addop,
                    )
                    if not RED_POOL:
                        dve_chain.append(red)
                    nc.scalar.activation(
                        out=vp[:, cs, HPC * E : W],
                        in_=a_k[:, hs].rearrange("p (c h) -> p c h", h=HPC),
                        func=mybir.ActivationFunctionType.Exp,
                        scale=SCALE,
                    )
                    if EW_FULL:
                        # materialize e_w across E so the vp multiply reads
                        # unit-stride (ACT has slack; DVE broadcast reads
                        # may be slow on HW)
                        nc.scalar.activation(
                            out=ew_f[:, cs, :].rearrange(
                                "p c (h e) -> p c h e", e=E
                            ),
                            in_=a_k[:, hs]
                            .rearrange("p (c h) -> p c h", h=HPC)
                            .unsqueeze(3)
                            .to_broadcast([P, 2 * GRAN, HPC, E]),
                            func=mybir.ActivationFunctionType.Exp,
                            scale=SCALE,
                        )

                def vp_group(g):
                    """vp = [V * e_w] for the group's chunks (e_w is read
                    back from the den columns the exp wrote)."""
                    nch = 2 * GRAN
                    cs = slice(nch * g, nch * (g + 1))
                    if EW_FULL:
                        ew_b = ew_f[:, cs, :].rearrange(
                            "p c (h e) -> p c h e", e=E
                        )
                    else:
                        ew_b = (
                            vp[:, cs, HPC * E : W]
                            .unsqueeze(3)
                            .to_broadcast([P, nch, HPC, E])
                        )
                    eng = nc.gpsimd if g in VP_POOL else nc.vector
                    vs = slice(0, nch) if DIAG_SMALL_IO else cs
                    v = eng.tensor_tensor(
                        out=vp[:, cs, 0 : HPC * E].rearrange(
                            "p c (h e) -> p c h e", e=E
                        ),
                        in0=v_t[:, vs, :].rearrange("p c (h e) -> p c h e", e=E),
                        in1=ew_b,
                        op=mult,
                    )
                    if g not in VP_POOL:
                        dve_chain.append(v)

                BANK = 512  # f32 elements per PSUM bank
                psc_of = {}  # pair -> psum tile

                def total_tri(c):
                    """block-total row + tri matmul for chunk c."""
                    w, r = divmod(c, RPB)
                    if r == 0:
                        ps_tot[w] = pt_pool.tile(
                            [RPB, W], f32, name="ps_tot", tag="ptot"
                        )
                    nc.tensor.matmul(
                        ps_tot[w][:, :],
                        sel[:, RPB - 1 - r : 2 * RPB - 1 - r],
                        vp[:, c, :],
                        start=(r == 0),
                        stop=(r == RPB - 1),
                    )
                    if r == RPB - 1:
                        t, b = divmod(w, BPT)
                        nc.scalar.copy(
                            out=tot_cat[t][32 * b : 32 * b + RPB, :],
                            in_=ps_tot[w],
                        )
                    cp, j = divmod(c, 2)
                    if j == 0:
                        psc_of[cp] = pc_pool.tile(
                            [P, 2, BANK], f32, name="psc", tag="psc"
                        )
                    blk = psc_of[cp][:, j, 0:W]
                    nc.tensor.matmul(
                        blk, triu[:, :], vp[:, c, :], start=True, stop=(c == 0)
                    )

                def prefixes(c):
                    """exclusive-prefix matmuls for chunk c against the padded
                    block-total stacks (emitted after the block copy so the
                    in-order PE queue never waits on a matmul behind it)."""
                    if c == 0:
                        return
                    cp, j = divmod(c, 2)
                    blk = psc_of[cp][:, j, 0:W]
                    last_w = (c - 1) // RPB  # last block with any prefix bit
                    for t in range(2):
                        if last_w < BPT * t:
                            break
                        b = min(last_w - BPT * t, BPT - 1)
                        kk = 32 * b + RPB
                        nc.tensor.matmul(
                            blk,
                            su_cat[t][0:kk, c : c + 1].to_broadcast([kk, P]),
                            tot_cat[t][0:kk, :],
                            start=False,
                            stop=(t == 1 or last_w < BPT),
                        )

                def norm_pair(cp):
                    """One reciprocal of the den columns and one broadcast
                    multiply to normalize; then store the pair."""
                    psc = psc_of.pop(cp)
                    c0 = 2 * cp
                    hs = slice(c0 * HPC, (c0 + 2) * HPC)
                    dve_chain.append(nc.vector.reciprocal(
                        out=r_all[:, hs].rearrange("p (c h) -> p c h", h=HPC),
                        in_=psc[:, :, HPC * E : W],
                    ))
                    r_b = (
                        r_all[:, hs]
                        .rearrange("p (c h) -> p c h", h=HPC)
                        .unsqueeze(3)
                        .to_broadcast([P, 2, HPC, E])
                    )
                    dve_chain.append(nc.vector.tensor_tensor(
                        out=o_t[:, c0 : c0 + 2, :].rearrange(
                            "p c (h e) -> p c h e", e=E
                        ),
                        in0=psc[:, :, 0 : HPC * E].rearrange(
                            "p c (h e) -> p c h e", e=E
                        ),
                        in1=r_b,
                        op=mult,
                    ))
                    if (cp + 1) % ST_GROUP == 0:
                        cs = slice(2 * (cp + 1 - ST_GROUP), 2 * (cp + 1))
                        dma_eng[ST_ENG].dma_start(out=ov[:, cs, :], in_=o_t[:, cs, :])

                NG = NP // LOAD_GROUP  # load groups
                next_v = 0

                def emit_v_upto(pair_target):
                    """Ensure V groups covering pairs <= pair_target are
                    emitted (into V_ENG's stream at the current point)."""
                    nonlocal next_v
                    while next_v < NG and next_v * LOAD_GROUP <= pair_target:
                        load_v(next_v)
                        next_v += 1

                SEQS = {
                    # K leading its pair's V enough for the a_k chain to
                    # finish by V's arrival (tuned pre-GRAN)
                    0: [("k", 0), ("v", 0), ("k", 1), ("v", 1), ("k", 2),
                        ("v", 2), ("k", 3), ("k", 4), ("v", 3), ("k", 5),
                        ("v", 4), ("k", 6), ("v", 5), ("k", 7), ("v", 6),
                        ("v", 7)],
                    # group-paced: K and V arrive in GRAN-sized blocks
                    1: [("k", 0), ("k", 1), ("v", 0), ("v", 1), ("k", 2),
                        ("k", 3), ("v", 2), ("v", 3), ("k", 4), ("k", 5),
                        ("v", 4), ("v", 5), ("k", 6), ("k", 7), ("v", 6),
                        ("v", 7)],
                    # strict alternate, K one pair ahead
                    2: [("k", 0), ("k", 1), ("v", 0), ("k", 2), ("v", 1),
                        ("k", 3), ("v", 2), ("k", 4), ("v", 3), ("k", 5),
                        ("v", 4), ("k", 6), ("v", 5), ("k", 7), ("v", 6),
                        ("v", 7)],
                }
                if DIAG_SMALL_IO:
                    load_k(0)
                    load_v(0)
                    next_v = NG
                elif TUNED_SEQ and V_ENG == K_ENG and LOAD_GROUP == 1:
                    for kind, p in SEQS[SEQ_VARIANT]:
                        (load_k if kind == "k" else load_v)(p)
                    next_v = NG
                elif V_ENG == K_ENG:
                    # same queue: interleave K/V groups, K one group ahead
                    load_k(0)
                    for g in range(1, NG):
                        load_k(g)
                        load_v(g - 1)
                    load_v(NG - 1)
                    next_v = NG
                else:
                    for g in range(NG):
                        load_k(g)
                    emit_v_upto(2)  # head start: cover the first ~3 pairs

                # Pool runs ahead of the DVE stream (its FIFO paces on the K
                # loads); each group's reduce is pulled ahead of the previous
                # group's vp, and norms lag NORM_LAG pairs so the PE->ACT->PE
                # round trip of a pair never stalls the DVE spine.
                for a in range(NP // AK_GRAN):
                    pool_head(a)
                red_exp(0)
                normed = 0
                NPG = NP // GRAN
                for g in range(NPG):
                    # V groups feeding this vp must be emitted before it so
                    # the dependency exists; +2 pairs of lead keeps the DMA
                    # queue ahead of the spine
                    emit_v_upto(GRAN * (g + 1) - 1 + 2)
                    vp_group(g)
                    if g + 1 < NPG:
                        red_exp(g + 1)
                    if WAVE_BATCH == 1:
                        for p in range(GRAN * g, GRAN * (g + 1)):
                            total_tri(2 * p)
                            total_tri(2 * p + 1)
                            prefixes(2 * p)
                            prefixes(2 * p + 1)
                    else:
                        # block copies land every RPB chunks; emit the whole
                        # group's totals first so every prefix sees its block
                        for c in range(2 * GRAN * g, 2 * GRAN * (g + 1)):
                            total_tri(c)
                        for c in range(2 * GRAN * g, 2 * GRAN * (g + 1)):
                            prefixes(c)
                    while normed <= GRAN * (g + 1) - 1 - NORM_LAG:
                        norm_pair(normed)
                        normed += 1
                while normed < NP:
                    norm_pair(normed)
                    normed += 1

            loop = tc.For_i(0, reps // un, 1) if reps > 1 else nullcontext()
            with loop:
                for _phase in range(un):
                    body(_phase)
                # Pin the static DVE order to the dataflow order
                for a, b in zip(dve_chain, dve_chain[1:]):
                    tile.add_dep_helper(
                        b.ins, a.ins, sync=False, reason="dve pipeline order"
                    )

    nc.compile()
    return nc


def _get_nc(reps=1):
    key = ("nc", reps, _cfg())
    if key not in _CACHE:
        _CACHE[key] = _build(reps)
    return _CACHE[key]


def _shard(inputs):
    keys = np.asarray(inputs["keys"], dtype=np.float32)
    values = np.asarray(inputs["values"], dtype=np.float32)
    w_score = np.asarray(inputs["w_score"], dtype=np.float32)
    wk = np.ascontiguousarray(w_score[E : 2 * E].reshape(1, E))
    in_maps = []
    for c in range(NCORES):
        b = c // (NCORES // B)
        h0 = HPC * (c % (NCORES // B))
        in_maps.append(
            {
                "k_in": np.ascontiguousarray(keys[b, :, h0 : h0 + HPC, :]),
                "v_in": np.ascontiguousarray(values[b, :, h0 : h0 + HPC, :]),
                "wk_in": wk,
            }
        )
    return in_maps


def _gather(results):
    out = np.empty((B, L, H, E), dtype=np.float32)
    for c in range(NCORES):
        b = c // (NCORES // B)
        h0 = HPC * (c % (NCORES // B))
        out[b, :, h0 : h0 + HPC, :] = np.asarray(results[c]["out"]).astype(
            np.float32
        )
    return out


def _run_sharded(inputs, reps=1, **kwargs):
    from concourse.bass_utils import run_bass_kernel_spmd

    nc = _get_nc(reps)
    in_maps = _shard(inputs)
    res = run_bass_kernel_spmd(nc, in_maps, core_ids=list(range(NCORES)), **kwargs)
    return res


def kernel(**inputs) -> np.ndarray:
    res = _run_sharded(inputs)
    return _gather(res.results)

